# revision 1
# baseline (speedup 1.0000x reference)
"""Multi-scale deformable attention (nearest sampling, sum over points) on
8 Trainium2 NeuronCores via Bass/Tile.

Sharding: the 240000 (batch*query) rows are split into 24 phases of 10000
queries (4 phases per batch); each of the 8 cores runs 3 phases. A phase
loads its batch's value slab into SBUF (twin channel-split f32 tables),
then streams query blocks: PE-transposes sampling (x,y) planes to a
[(head,level,point), query] layout, computes nearest-sample indices with an
exact float32 op chain matching the reference, gathers value rows with the
GpSimd ap_gather ucode instruction, reduces the 16 points per query on the
vector engine, PE-transposes back to [query, channels] and streams out.

No cross-core communication; inputs/outputs are sharded/assembled on host.
"""
import numpy as np

SPATIAL = [(64, 176), (32, 88), (16, 44), (8, 22)]
LVL_OFF = [0, 11264, 14080, 14784]
NKEY = 14960
BS, NQ = 6, 40000
QPP, PHASES, QB = 10000, 3, 256
N_CORES = 8
MAGIC = 12582912.0  # 1.5 * 2**23 : float32 round-to-nearest-even bias

_CACHE = {}


def _make_consts():
    c = np.zeros((128, 8), np.float32)
    for p in range(128):
        l = (p % 16) // 4
        h_l, w_l = SPATIAL[l]
        off = LVL_OFF[l]
        c[p, 0] = w_l
        c[p, 1] = h_l
        c[p, 2] = MAGIC - off
        c[p, 3] = off + w_l - 1
        c[p, 4] = h_l - 1
    return c


def _build_program(qpp, phases, qb, nkb):
    from concourse import bacc, tile, mybir, library_config

    F32 = mybir.dt.float32
    I16 = mybir.dt.int16
    assert qb % 128 == 0
    nblk = (qpp + qb - 1) // qb
    qc = qb // 128

    nc = bacc.Bacc("TRN2", target_bir_lowering=False, debug=False)
    values3 = nc.dram_tensor("values3", [phases, NKEY, 256], F32,
                             kind="ExternalInput")
    samp3 = nc.dram_tensor("samp3", [phases, qpp, 256], F32,
                           kind="ExternalInput")
    consts = nc.dram_tensor("consts", [128, 8], F32, kind="ExternalInput")
    ident_in = nc.dram_tensor("ident", [128, 128], F32, kind="ExternalInput")
    out_ext = nc.dram_tensor("out", [phases * qpp, 256], F32,
                             kind="ExternalOutput")

    with tile.TileContext(nc) as tc:
        with tc.tile_pool(name="tab", bufs=1) as tabp, \
             tc.tile_pool(name="cst", bufs=1) as cstp, \
             tc.tile_pool(name="rawv", bufs=3) as rawvp, \
             tc.tile_pool(name="sraw", bufs=3) as srawp, \
             tc.tile_pool(name="xy", bufs=2) as xyp, \
             tc.tile_pool(name="idx", bufs=2) as idxp, \
             tc.tile_pool(name="g", bufs=2) as gp, \
             tc.tile_pool(name="g2", bufs=1) as gp2, \
             tc.tile_pool(name="r", bufs=2) as rp, \
             tc.tile_pool(name="ost", bufs=3) as ostp, \
             tc.tile_pool(name="pst", bufs=2, space="PSUM") as pst, \
             tc.tile_pool(name="psxy", bufs=2, space="PSUM") as psxy:

            tabA = tabp.tile([128, NKEY], F32, tag="tabA")
            tabB = tabp.tile([128, NKEY], F32, tag="tabB")
            cst = cstp.tile([128, 8], F32, tag="cst")
            idn = cstp.tile([128, 128], F32, tag="idn")
            nc.sync.dma_start(out=cst[:], in_=consts[:])
            nc.sync.dma_start(out=idn[:], in_=ident_in[:])
            W = cst[:, 0:1]
            Hh = cst[:, 1:2]
            XC = cst[:, 2:3]
            XHI = cst[:, 3:4]
            YHI = cst[:, 4:5]

            nc.gpsimd.load_library(library_config.ap_gather)

            for ph in range(phases):
                for kk in range(nkb):
                    k0 = kk * 128
                    n = min(128, NKEY - k0)
                    raw = rawvp.tile([128, 256], F32, tag="rawv")
                    nc.sync.dma_start(out=raw[:n, :],
                                      in_=values3[ph, k0:k0 + n, :])
                    movA = raw[:n, 0:128]
                    movB = raw[:n, 128:256]
                    pA = pst.tile([128, 128], F32, tag="tA")
                    pB = pst.tile([128, 128], F32, tag="tB")
                    nc.tensor.transpose(pA[:, :n], movA, idn[:n, :n])
                    nc.tensor.transpose(pB[:, :n], movB, idn[:n, :n])
                    nc.scalar.copy(out=tabA[:, k0:k0 + n], in_=pA[:, :n])
                    nc.scalar.copy(out=tabB[:, k0:k0 + n], in_=pB[:, :n])

                for blk in range(nblk):
                    q0 = min(blk * qb, qpp - qb)
                    xps = psxy.tile([128, qb], F32, tag="xps")
                    yps = psxy.tile([128, qb], F32, tag="yps")
                    for c in range(qc):
                        raw = srawp.tile([128, 256], F32, tag="sraw")
                        nc.sync.dma_start(
                            out=raw[:],
                            in_=samp3[ph, q0 + c * 128: q0 + (c + 1) * 128, :])
                        movX = raw[:].rearrange(
                            "q (e x) -> q e x", x=2)[:, :, 0]
                        movY = raw[:].rearrange(
                            "q (e x) -> q e x", x=2)[:, :, 1]
                        nc.tensor.transpose(
                            xps[:, c * 128:(c + 1) * 128], movX, idn[:])
                        nc.tensor.transpose(
                            yps[:, c * 128:(c + 1) * 128], movY, idn[:])
                    xf = xyp.tile([128, qb], F32, tag="xf")
                    yf = xyp.tile([128, qb], F32, tag="yf")
                    nc.scalar.activation(
                        out=xf[:], in_=xps[:],
                        func=mybir.ActivationFunctionType.Copy,
                        scale=2.0, bias=-1.0)
                    nc.scalar.activation(
                        out=yf[:], in_=yps[:],
                        func=mybir.ActivationFunctionType.Copy,
                        scale=2.0, bias=-1.0)
                    A = mybir.AluOpType
                    nc.vector.tensor_scalar(out=xf[:], in0=xf[:],
                                            scalar1=1.0, scalar2=W,
                                            op0=A.add, op1=A.mult)
                    nc.vector.tensor_scalar(out=xf[:], in0=xf[:],
                                            scalar1=0.5, scalar2=-0.5,
                                            op0=A.mult, op1=A.add)
                    nc.vector.tensor_scalar(out=xf[:], in0=xf[:],
                                            scalar1=MAGIC, scalar2=XC,
                                            op0=A.add, op1=A.subtract)
                    nc.vector.tensor_scalar(out=xf[:], in0=xf[:],
                                            scalar1=XHI, scalar2=None,
                                            op0=A.min)
                    nc.vector.tensor_scalar(out=yf[:], in0=yf[:],
                                            scalar1=1.0, scalar2=Hh,
                                            op0=A.add, op1=A.mult)
                    nc.vector.tensor_scalar(out=yf[:], in0=yf[:],
                                            scalar1=0.5, scalar2=-0.5,
                                            op0=A.mult, op1=A.add)
                    nc.vector.tensor_scalar(out=yf[:], in0=yf[:],
                                            scalar1=MAGIC, scalar2=MAGIC,
                                            op0=A.add, op1=A.subtract)
                    nc.vector.tensor_scalar(out=yf[:], in0=yf[:],
                                            scalar1=YHI, scalar2=None,
                                            op0=A.min)
                    idx = idxp.tile([128, qb], I16, tag="idx")
                    nc.vector.scalar_tensor_tensor(
                        out=idx[:], in0=yf[:], scalar=W, in1=xf[:],
                        op0=A.mult, op1=A.add)
                    # duplicate each head's stream into the two 16-partition
                    # groups covering its 32 channels
                    idxA = idxp.tile([128, qb], I16, tag="idxA")
                    idxB = idxp.tile([128, qb], I16, tag="idxB")
                    for hh in range(4):
                        for dd in range(2):
                            p0 = 32 * hh + 16 * dd
                            nc.sync.dma_start(
                                out=idxA[p0:p0 + 16, :],
                                in_=idx[16 * hh:16 * hh + 16, :])
                            nc.sync.dma_start(
                                out=idxB[p0:p0 + 16, :],
                                in_=idx[64 + 16 * hh:64 + 16 * hh + 16, :])
                    gA = gp.tile([128, qb * 16], F32, tag="gA")
                    nc.gpsimd.ap_gather(
                        out_ap=gA[:], in_ap=tabA[:].unsqueeze(2),
                        idxs_ap=idxA[:], channels=128, num_elems=NKEY,
                        d=1, num_idxs=qb * 16)
                    gB = gp2.tile([128, qb * 16], F32, tag="gB")
                    nc.gpsimd.ap_gather(
                        out_ap=gB[:], in_ap=tabB[:].unsqueeze(2),
                        idxs_ap=idxB[:], channels=128, num_elems=NKEY,
                        d=1, num_idxs=qb * 16)
                    rA = rp.tile([128, qb], F32, tag="rA")
                    rB = rp.tile([128, qb], F32, tag="rB")
                    nc.vector.tensor_reduce(
                        out=rA[:],
                        in_=gA[:].rearrange("p (q s) -> p q s", s=16),
                        axis=mybir.AxisListType.X, op=A.add)
                    nc.vector.tensor_reduce(
                        out=rB[:],
                        in_=gB[:].rearrange("p (q s) -> p q s", s=16),
                        axis=mybir.AxisListType.X, op=A.add)
                    for c in range(qc):
                        tA = pst.tile([128, 128], F32, tag="tA")
                        tB = pst.tile([128, 128], F32, tag="tB")
                        nc.tensor.transpose(
                            tA[:], rA[:, c * 128:(c + 1) * 128], idn[:])
                        nc.tensor.transpose(
                            tB[:], rB[:, c * 128:(c + 1) * 128], idn[:])
                        ost = ostp.tile([128, 256], F32, tag="ost")
                        nc.scalar.copy(out=ost[:, 0:128], in_=tA[:])
                        nc.scalar.copy(out=ost[:, 128:256], in_=tB[:])
                        row0 = ph * qpp + q0 + c * 128
                        nc.sync.dma_start(
                            out=out_ext[row0:row0 + 128, :], in_=ost[:])
    return nc


def _compile_spmd(nc, n_cores):
    """Compile-once runner based on concourse.bass2jax.run_bass_via_pjrt."""
    import jax
    from jax.sharding import Mesh, PartitionSpec, NamedSharding
    try:
        from jax.experimental.shard_map import shard_map
    except ImportError:
        from jax.shard_map import shard_map
    from concourse import mybir
    from concourse.bass2jax import (
        install_neuronx_cc_hook, _bass_exec_p, partition_id_tensor)

    install_neuronx_cc_hook()
    if not nc.is_finalized():
        nc.finalize()
    partition_name = (nc.partition_id_tensor.name
                      if nc.partition_id_tensor else None)

    in_names, out_names, out_avals, zero_outs = [], [], [], []
    for alloc in nc.m.functions[0].allocations:
        if not isinstance(alloc, mybir.MemoryLocationSet):
            continue
        name = alloc.memorylocations[0].name
        if alloc.kind == "ExternalInput":
            if name != partition_name:
                in_names.append(name)
        elif alloc.kind == "ExternalOutput":
            out_names.append(name)
            shape = tuple(alloc.tensor_shape)
            dtype = mybir.dt.np(alloc.dtype)
            out_avals.append(jax.core.ShapedArray(shape, dtype))
            zero_outs.append(np.zeros(shape, dtype))
    n_params = len(in_names)
    all_in_names = (in_names + out_names
                    + ([partition_name] if partition_name else []))

    def _body(*args):
        operands = list(args)
        if partition_name is not None:
            operands.append(partition_id_tensor())
        outs = _bass_exec_p.bind(
            *operands,
            out_avals=tuple(out_avals),
            in_names=tuple(all_in_names),
            out_names=tuple(out_names),
            lowering_input_output_aliases=(),
            sim_require_finite=True,
            sim_require_nnan=True,
            nc=nc,
        )
        return tuple(outs)

    devices = jax.devices()[:n_cores]
    mesh = Mesh(np.asarray(devices), ("core",))
    in_specs = (PartitionSpec("core"),) * (n_params + len(out_names))
    out_specs = (PartitionSpec("core"),) * len(out_names)
    sharded = jax.jit(
        shard_map(_body, mesh=mesh, in_specs=in_specs,
                  out_specs=out_specs, check_rep=False),
        keep_unused=True,
    )
    sh = NamedSharding(mesh, PartitionSpec("core"))

    def prep(in_maps):
        staged = [
            jax.device_put(
                np.concatenate([m[name] for m in in_maps], axis=0), sh)
            for name in in_names
        ]
        staged += [
            jax.device_put(np.concatenate([z] * n_cores, axis=0), sh)
            for z in zero_outs
        ]
        return staged

    def run(staged):
        return sharded(*staged)
    return run, prep, in_names, out_names


def _get_compiled():
    if "run" not in _CACHE:
        nkb = (NKEY + 127) // 128
        nc = _build_program(QPP, PHASES, QB, nkb)
        run, prep, in_names, out_names = _compile_spmd(nc, N_CORES)
        _CACHE.update(run=run, prep=prep, in_names=in_names,
                      out_names=out_names)
    return _CACHE


def _shard_inputs(value, sampling_locations):
    vflat = np.ascontiguousarray(value.reshape(BS, NKEY, 256))
    sflat = np.ascontiguousarray(
        sampling_locations.reshape(BS, NQ, 256))
    consts = _make_consts()
    ident = np.eye(128, dtype=np.float32)
    in_maps = []
    for c in range(N_CORES):
        v3 = np.empty((PHASES, NKEY, 256), np.float32)
        s3 = np.empty((PHASES, QPP, 256), np.float32)
        for j in range(PHASES):
            g = c * PHASES + j
            b = (g * QPP) // NQ
            q0 = (g * QPP) % NQ
            v3[j] = vflat[b]
            s3[j] = sflat[b, q0:q0 + QPP]
        in_maps.append({"values3": v3, "samp3": s3,
                        "consts": consts, "ident": ident})
    return in_maps


def kernel(value, value_spatial_shapes, sampling_locations):
    import jax
    value = np.asarray(value, np.float32)
    sampling_locations = np.asarray(sampling_locations, np.float32)
    cc = _get_compiled()
    in_maps = _shard_inputs(value, sampling_locations)
    staged = cc["prep"](in_maps)
    outs = cc["run"](staged)
    jax.block_until_ready(outs)
    full = np.asarray(outs[0])                 # (8*30000, 256)
    return np.ascontiguousarray(full.reshape(BS, NQ, 256))

